# revision 28
# baseline (speedup 1.0000x reference)
"""Trainium2 Bass kernel for nn_LuenbergerLDS (B=32, T=2048, N=512, M=512).

Math: the reference is a diagonal complex linear recurrence followed by a
projection; since d == 1 the whole module is a causal LTI SIMO filter
    y[t, b, m] = sum_{j>=0} H[j, m] * x[t - j, b] + Do[m]
with impulse response H computed on host in float64 from (lam, Winv, C, D).

Key structure exploited here (vs the previous all-direct-FIR version):
the FIR tail H[LH:LH+WT] is numerically low-rank (singular values fall
below 1e-3 of ||y|| by index ~48), so it is factored H_tail ~= U @ V
(rank R) via SVD on host.  The device then computes, per output chunk of
128 timesteps, a SINGLE 128x512 fp16 matmul whose stationary operand
stacks [x head lags (LH=80) ; tail coefficients z (R=48)] and whose
moving operand stacks [H_head ; V].  z itself comes from a cheap "basis
conv" stage: 4 matmuls per 512-timestep superchunk contracting lag tiles
of U against diagonal (Toeplitz) slices of x.

The z coefficients are written (fp32->fp16 copy) into partitions 80..127
of the SAME per-batch diagonal x-buffer that serves the head lags in
partitions 0..79, so the output matmul's stationary operand is a plain
contiguous slice.  The diagonal buffer xsh[p, v] = xpad[v + p] makes
both the conv moving slices and the head/stacked stationary slices
simple strided views.

dtype: fp16 everywhere on the PE (10 mantissa bits; measured end-to-end
error 6.7e-4 of max|y| vs 2e-2 tolerance).  PSUM accumulates fp32.
Output is written fp16 and upcast on host (adds Do there too).

Per core (4 batches): 16 conv matmuls + 16 output matmuls per batch =
128 matmuls-512 total (vs 452 in the previous version).  Output DMAs are
staged 4 output tiles per dma_start to amortize DGE cost; evacuation
copies rotate across DVE/Pool/Activation engines.
"""

import os
import sys

sys.path.insert(0, "/opt/trn_rl_repo")

import numpy as np

# problem dims (hardcoded per harness contract)
B, T, N, M = 32, 2048, 512, 512
NCORES = 8
BLOC = B // NCORES          # batches per core
LH = 64                     # direct head lags [0, LH); partition-quadrant aligned
R = 128 - LH                # tail rank (stacked into the same 128 contraction)
NTILE = 4                   # conv lag tiles of 128 -> tail window
WT = NTILE * 128            # tail lags [LH, LH+WT)
RPAD = LH + WT - 1          # 591: left zero pad of x
NV = T + 512                # diag buffer v-range [0, NV)
XROWS = NV + 127            # xpad rows so the diag load never reads OOB
NCH = T // 128              # output chunks per batch
NSC = T // 512              # superchunks per batch

MODE = os.environ.get("K_MODE", "f16")  # f16 | f8far
NLAG = NTILE  # kept for test.py cache-key compatibility
NV8 = 2176                  # fp8 far-tail diag buffer v-range (f8far mode)
XROWS8 = NV8 + 127


def build_program(mode=MODE):
    import concourse.tile as tile
    from concourse import bacc, mybir

    f16 = mybir.dt.float16
    f32 = mybir.dt.float32

    f8 = mybir.dt.float8e4
    f8far = mode == "f8far"
    nu16 = 2 if f8far else NTILE    # lag tiles kept fp16

    nc = bacc.Bacc("TRN2", target_bir_lowering=False, debug=False)
    xpad_t = nc.dram_tensor("xpad", [BLOC, XROWS], f16, kind="ExternalInput")
    hcomb_t = nc.dram_tensor("hcomb", [128, M], f16, kind="ExternalInput")
    ucomb_t = nc.dram_tensor("ucomb", [128, nu16 * R], f16, kind="ExternalInput")
    if f8far:
        xpad8_t = nc.dram_tensor("xpad8", [BLOC, XROWS8], f8, kind="ExternalInput")
        ucomb8_t = nc.dram_tensor("ucomb8", [128, 2 * R], f8, kind="ExternalInput")
    y_t = nc.dram_tensor("y", [BLOC, T, M], f16, kind="ExternalOutput")

    VCH = 1024                  # v-granularity of xsh loads (2KB descriptors)
    with tile.TileContext(nc) as tc:
        with (
            tc.tile_pool(name="xsh", bufs=1) as xsh_pool,
            tc.tile_pool(name="w", bufs=1) as wpool,
            tc.tile_pool(name="psum", bufs=1, space="PSUM") as psum_pool,
            tc.tile_pool(name="stage", bufs=1) as stage_pool,
        ):
            # ---- load plan (critical-first, 3 queues round-robin) ----
            xsh = []
            xsh8 = []
            for b in range(BLOC):
                t_ = xsh_pool.tile([128, NV], f16, tag=f"xshb{b}", name=f"xsh{b}")
                xsh.append(t_)
                if f8far:
                    t8 = xsh_pool.tile(
                        [128, NV8], f8, tag=f"xsh8b{b}", name=f"xsh8{b}"
                    )
                    xsh8.append(t8)
            ucomb_sb = wpool.tile([128, nu16 * R], f16, tag="ucomb", name="ucomb_sb")
            if f8far:
                ucomb8_sb = wpool.tile([128, 2 * R], f8, tag="ucomb8", name="ucomb8_sb")
            hcomb_sb = wpool.tile([128, M], f16, tag="hcomb", name="hcomb_sb")

            engines = [nc.sync, nc.scalar, nc.gpsimd]
            ei = 0

            def dma(eng, out_ap, in_ap):
                eng.dma_start(out=out_ap, in_=in_ap)

            def load_xchunk(b, v0, eng):
                nvc = min(VCH, NV - v0)
                in_ap = xpad_t.ap().copy()
                from bass_rust import VecI64Pair
                in_ap.ap = VecI64Pair([[1, 128], [1, nvc]])
                in_ap.offset = b * XROWS + v0
                dma(eng, xsh[b][:, v0 : v0 + nvc], in_ap)

            def load_x8chunk(b, v0, eng):
                nvc = min(VCH, NV8 - v0)
                in_ap = xpad8_t.ap().copy()
                from bass_rust import VecI64Pair
                in_ap.ap = VecI64Pair([[1, 128], [1, nvc]])
                in_ap.offset = b * XROWS8 + v0
                dma(eng, xsh8[b][:, v0 : v0 + nvc], in_ap)

            # Critical-path load plan.  The first conv group (b0, s0) reads
            # only v [0, 896) + ucomb, so b0's first 1024-v chunk rides the
            # sync HWDGE queue right behind tiny ucomb; b0's second chunk
            # goes on scalar (free once its ACT_TABLE_LOAD finishes);
            # b2/b3 ride the slower SWDGE queue -- they aren't needed until
            # the second schedule wave.
            dma(nc.sync, ucomb_sb[:], ucomb_t.ap())
            if f8far:
                dma(nc.sync, ucomb8_sb[:], ucomb8_t.ap())
            dma(nc.scalar, hcomb_sb[:], hcomb_t.ap())
            for b, v0, eng in (
                (0, 0, nc.sync),
                (0, 1024, nc.scalar),
                (1, 0, nc.sync),
                (1, 1024, nc.scalar),
                (0, 2048, nc.sync),
                (1, 2048, nc.scalar),
            ):
                load_xchunk(b, v0, eng)
            for v0 in (0, 1024, 2048):
                for b in (2, 3):
                    load_xchunk(b, v0, nc.gpsimd)
            if f8far:
                for b in range(BLOC):
                    for v0 in (0, 1024):
                        load_x8chunk(b, v0, nc.gpsimd)

            # ---- compute ----
            # NOTE: GpSimd (Pool) cannot access PSUM, so evacuation copies
            # rotate across DVE + Activation only; y-write DMA issue goes to
            # SP + Pool to keep those two engines free for copies.
            evac_engines = [nc.vector, nc.scalar]
            ydma_engines = [nc.sync, nc.gpsimd]
            ci = 0
            yi = 0
            def zcopy(b, s, zt):
                nonlocal ci
                eng = evac_engines[ci % 2]
                ci += 1
                w0 = 512 + 512 * s
                if eng is nc.scalar:
                    eng.copy(xsh[b][LH:128, w0 : w0 + 512], zt[LH:128, :M])
                else:
                    eng.tensor_copy(xsh[b][LH:128, w0 : w0 + 512], zt[LH:128, :M])

            from bass_rust import VecI64Pair

            prevzt = {}

            def conv_group(b, s):
                # write z^T into PSUM partitions 64..127 directly
                # (tile_position col=64) so the copy to SBUF partitions
                # 64..127 never crosses partitions.  All PSUM goes through
                # one [128, 2M] (2-bank) tag so conv and out phases share
                # the 8 banks without overcommitting.
                zt = psum_pool.tile([128, M], f32, tag="zt", bufs=2, name="zt")
                if f8far:
                    # far lag tiles l=3 (i=0) and l=2 (i=1) in one fp8
                    # DoubleRow matmul: rhs is an overlapping (2, 512) view
                    # of the fp8 diag buffer (windows 512s and 512s+128).
                    rhs8 = xsh8[b][:, 0:512].copy()
                    rhs8.ap = VecI64Pair([[NV8, 128], [128, 2], [1, 512]])
                    rhs8.offset = 512 * s
                    nc.tensor.matmul(
                        zt[LH:128, :M],
                        lhsT=ucomb8_sb[:].rearrange("p (i q) -> p i q", i=2),
                        rhs=rhs8,
                        start=True,
                        stop=False,
                        perf_mode=mybir.MatmulPerfMode.DoubleRow,
                    )
                    for i, l in enumerate((1, 0)):
                        v = 384 + 512 * s - 128 * l
                        nc.tensor.matmul(
                            zt[LH:128, :M],
                            lhsT=ucomb_sb[:, l * R : (l + 1) * R],
                            rhs=xsh[b][:, v : v + 512],
                            start=False,
                            stop=(i == 1),
                        )
                else:
                    for i, l in enumerate(range(NTILE - 1, -1, -1)):
                        v = 384 + 512 * s - 128 * l
                        nc.tensor.matmul(
                            zt[LH:128, :M],
                            lhsT=ucomb_sb[:, l * R : (l + 1) * R],
                            rhs=xsh[b][:, v : v + 512],
                            start=(i == 0),
                            stop=(i == NTILE - 1),
                        )
                # CRITICAL ORDER: zcopy(b, s) overwrites diag-buffer cells
                # (partitions 64..127, window [512+512s, 1024+512s)) that
                # conv(b, s+1) still reads as x data, so zcopy(b, s-1) is
                # emitted only now (after this group's matmuls), and
                # zcopy(b, NSC-1) right after its own group.
                if s > 0:
                    zcopy(b, s - 1, prevzt[b])
                prevzt[b] = zt
                if s == NSC - 1:
                    zcopy(b, s, zt)

            def out_pair(b, c0, osb, off, split=False):
                # two output-chunk matmuls into one 2-bank PSUM tile, then a
                # single fused (2-chunk-wide) evacuation copy -- halves the
                # per-op fixed cost on the two PSUM-capable engines.  With
                # split=True the two halves are copied by both engines in
                # parallel instead (shorter latency for the kernel tail).
                nonlocal ci
                pp = psum_pool.tile([128, 2 * M], f32, tag="po", bufs=3, name="pp")
                for k in (0, 1):
                    w = 512 + 128 * (c0 + k)
                    nc.tensor.matmul(
                        pp[:, k * M : (k + 1) * M],
                        lhsT=xsh[b][:, w : w + 128],
                        rhs=hcomb_sb[:],
                        start=True,
                        stop=True,
                    )
                if split:
                    nc.vector.tensor_copy(osb[:, off : off + M], pp[:, :M])
                    nc.scalar.copy(osb[:, off + M : off + 2 * M], pp[:, M:])
                else:
                    eng = evac_engines[ci % 2]
                    ci += 1
                    if eng is nc.scalar:
                        eng.copy(osb[:, off : off + 2 * M], pp[:])
                    else:
                        eng.tensor_copy(osb[:, off : off + 2 * M], pp[:])

            def out_group(b, g, ydma_eng=None, tail=False):
                # one staged y write of 4 output chunks (2 fused pairs)
                nonlocal yi
                tag, bufs = ("osbt", 2) if tail else ("osb", 6)
                osb = stage_pool.tile([128, 4 * M], f16, tag=tag, bufs=bufs, name=tag)
                out_pair(b, 4 * g, osb, 0, split=tail)
                out_pair(b, 4 * g + 2, osb, 2 * M, split=tail)
                dst = y_t.ap().copy()
                dst.ap = VecI64Pair([[M, 128], [128 * M, 4], [1, M]])
                dst.offset = b * T * M + g * 512 * M
                eng = ydma_eng or ydma_engines[yi % 2]
                yi += 1
                eng.dma_start(out=dst, in_=osb[:])

            def tail_group(b, chunks, ydma_eng):
                nonlocal ci
                osb = stage_pool.tile(
                    [128, 4 * M], f16, tag="osbt", bufs=2, name="osbt"
                )
                pp = psum_pool.tile([128, 2 * M], f32, tag="po", bufs=3, name="pp")
                for k, c in enumerate(chunks):
                    w = 512 + 128 * c
                    nc.tensor.matmul(
                        pp[:, k * M : (k + 1) * M],
                        lhsT=xsh[b][:, w : w + 128],
                        rhs=hcomb_sb[:],
                        start=True,
                        stop=True,
                    )
                    eng = evac_engines[(ci + k) % 2]
                    if eng is nc.scalar:
                        eng.copy(osb[:, k * M : (k + 1) * M], pp[:, k * M : (k + 1) * M])
                    else:
                        eng.tensor_copy(
                            osb[:, k * M : (k + 1) * M], pp[:, k * M : (k + 1) * M]
                        )
                ci += len(chunks)
                dst = y_t.ap().copy()
                dst.ap = VecI64Pair([[M, 128], [128 * M, len(chunks)], [1, M]])
                dst.offset = b * T * M + chunks[0] * 128 * M
                ydma_eng.dma_start(out=dst, in_=osb[:, : len(chunks) * M])

            # Schedule: conv(pair1) -> [out(pair1) interleaved with
            # conv(pair2)] (that section is PE-bound: evac demand of the out
            # groups fits beside the conv groups' zcopies) -> out(pair2)
            # with a short-latency tail.
            for b in (0, 1):
                for s in range(NSC):
                    conv_group(b, s)
            og = [(b, g) for b in (0, 1) for g in range(NCH // 4)]
            cg = [(b, s) for b in (2, 3) for s in range(NSC)]
            for i in range(len(og)):
                out_group(*og[i])
                conv_group(*cg[i])
            for g in range(NCH // 4):
                out_group(2, g)
                if g < 3:
                    out_group(3, g)
            # kernel tail: last superchunk of the last batch goes out as a
            # 2-chunk group plus two 1-chunk groups so the final transfers
            # are small and issue on the idle sync/scalar HW queues.
            tail_group(3, (12, 13), nc.gpsimd)
            tail_group(3, (14,), nc.scalar)
            tail_group(3, (15,), nc.sync)

    nc.compile()
    return nc


def host_weights(lnl_re, lnl_im, W_r, W_i, C, D, Do, mode=MODE):
    """Impulse response head + SVD-factored tail, float64 math."""
    lnl = lnl_re.astype(np.float64) + 1j * lnl_im.astype(np.float64)
    W = W_r.astype(np.float64) + 1j * W_i.astype(np.float64)
    Winv = np.linalg.inv(W)
    A_re = np.ascontiguousarray(Winv.real.T) @ C.astype(np.float64)
    A_im = np.ascontiguousarray(Winv.imag.T) @ C.astype(np.float64)
    j = np.arange(LH + WT, dtype=np.float64)
    P = np.exp(np.outer(j, lnl))
    H = P.real @ A_re - P.imag @ A_im                 # (LH+WT, M)
    H[0] += D[0].astype(np.float64)

    Hh = H[:LH]
    U, S, Vt = np.linalg.svd(H[LH:], full_matrices=False)
    sq = np.sqrt(S[:R])
    Uf = U[:, :R] * sq                                # (WT, R)
    Vf = sq[:, None] * Vt[:R]                         # (R, M)

    hcomb = np.concatenate([Hh[::-1], Vf], axis=0).astype(np.float16)
    uflip = Uf.reshape(NTILE, 128, R)[:, ::-1, :]     # [l, p, q], p-flipped
    if mode == "f8far":
        import ml_dtypes

        ucomb = (
            uflip[:2].transpose(1, 0, 2).reshape(128, 2 * R).astype(np.float16)
        )
        # DoubleRow weights: i=0 -> lag tile l=3, i=1 -> l=2
        u8 = np.stack([uflip[3], uflip[2]], axis=1).reshape(128, 2 * R)
        out = {
            "hcomb": np.ascontiguousarray(hcomb),
            "ucomb": np.ascontiguousarray(ucomb),
            "ucomb8": np.ascontiguousarray(u8.astype(ml_dtypes.float8_e4m3)),
        }
        return out
    ucomb = (
        uflip.transpose(1, 0, 2).reshape(128, NTILE * R).astype(np.float16)
    )
    return {
        "hcomb": np.ascontiguousarray(hcomb),
        "ucomb": np.ascontiguousarray(ucomb),
    }


def make_in_maps(x, weights):
    x16 = x[:, :, 0].astype(np.float16)               # (B, T)
    if MODE == "f8far":
        import ml_dtypes

        x8 = x[:, :, 0].astype(ml_dtypes.float8_e4m3)
    in_maps = []
    for c in range(NCORES):
        xpad = np.zeros((BLOC, XROWS), np.float16)
        xpad[:, RPAD : RPAD + T] = x16[c * BLOC : (c + 1) * BLOC]
        im = dict(weights)
        im["xpad"] = xpad
        if MODE == "f8far":
            import ml_dtypes

            xpad8 = np.zeros((BLOC, XROWS8), ml_dtypes.float8_e4m3)
            n8 = XROWS8 - RPAD
            xpad8[:, RPAD:] = x8[c * BLOC : (c + 1) * BLOC, :n8]
            im["xpad8"] = xpad8
        in_maps.append(im)
    return in_maps


_prog_cache = {}


def kernel(x, lnl_re, lnl_im, W_r, W_i, C, D, Do):
    from concourse.bass_utils import run_bass_kernel_spmd

    x = np.asarray(x)
    lnl_re, lnl_im = np.asarray(lnl_re), np.asarray(lnl_im)
    W_r, W_i = np.asarray(W_r), np.asarray(W_i)
    C, D, Do = np.asarray(C), np.asarray(D), np.asarray(Do)

    key = (NLAG, MODE)
    if key not in _prog_cache:
        _prog_cache[key] = build_program()
    nc = _prog_cache[key]

    weights = host_weights(lnl_re, lnl_im, W_r, W_i, C, D, Do)
    in_maps = make_in_maps(x, weights)
    res = run_bass_kernel_spmd(nc, in_maps, core_ids=list(range(NCORES)))
    y = np.concatenate([res.results[i]["y"] for i in range(NCORES)], axis=0)
    y = y.astype(np.float32) + Do.astype(np.float32)[None, None, :]
    return np.ascontiguousarray(y)


# revision 30
# speedup vs baseline: 1.0842x; 1.0842x over previous
"""Trainium2 Bass kernel for nn_LuenbergerLDS (B=32, T=2048, N=512, M=512).

Math: the reference is a diagonal complex linear recurrence followed by a
projection; since d == 1 the whole module is a causal LTI SIMO filter
    y[t, b, m] = sum_{j>=0} H[j, m] * x[t - j, b] + Do[m]
with impulse response H computed on host in float64 from (lam, Winv, C, D).

Structure: the FIR tail H[LH:LH+WT] is numerically low-rank (singular
values fall below 1e-3 of ||y|| by index ~48), so it is factored
H_tail ~= U @ V (rank R = 64) via SVD on host.  The device computes, per
output chunk of 128 timesteps, a SINGLE 128x512 fp16 matmul whose
stationary operand stacks [x head lags (LH=64) ; tail coefficients z
(R=64)] and whose moving operand stacks [H_head ; V].  z comes from a
cheap "basis conv" stage: 4 matmuls per 512-timestep superchunk
contracting lag tiles of U against diagonal (Toeplitz) slices of x.

The z coefficients are written (fp32->fp16 copy, PSUM partitions 64..127
via matmul tile_position) into partitions 64..127 of the SAME per-batch
diagonal x-buffer that serves the head lags in partitions 0..63, so the
output matmul's stationary operand is a plain contiguous slice.
ORDERING HAZARD: that overlay write lands on cells conv(b, s+1) still
reads as x data, so zcopy(b, s) must be emitted after conv(b, s+1); the
zt PSUM pool needs 3 buffers so the slot-reuser group sits 2 groups
behind the zcopy and the WAR wait never stalls the PE.

dtype fp16 on the PE (measured end-to-end error 6.2e-4 of max|y| vs
2e-2 tolerance); PSUM accumulates fp32; y is written fp16 and upcast on
host (Do added there).  Per core: 64 conv + 64 output matmuls.

Engine economy: only DVE + Activation can read PSUM, so evacuation
copies rotate across exactly those two; y writes are staged 4 chunks
per dma_start and issued from SP/Pool; loads are ordered critical-first
(b0/b1 ride the two HWDGE queues; b2/b3 queue FIFO behind on SWDGE).
"""

import os
import sys

sys.path.insert(0, "/opt/trn_rl_repo")

import numpy as np

# problem dims (hardcoded per harness contract)
B, T, N, M = 32, 2048, 512, 512
NCORES = 8
BLOC = B // NCORES          # batches per core
LH = 64                     # direct head lags [0, LH); partition-quadrant aligned
R = 128 - LH                # tail rank (stacked into the same 128 contraction)
NTILE = 4                   # conv lag tiles of 128 -> tail window
WT = NTILE * 128            # tail lags [LH, LH+WT)
RPAD = LH + WT - 1          # 575: left zero pad of x
NV = T + 512                # diag buffer v-range [0, NV)
XROWS = NV + 127            # xpad rows so the diag load never reads OOB
NCH = T // 128              # output chunks per batch
NSC = T // 512              # superchunks per batch

MODE = os.environ.get("K_MODE", "f16")  # f16 | f8far
NLAG = NTILE  # kept for test.py cache-key compatibility
NV8 = 2176                  # fp8 far-tail diag buffer v-range (f8far mode)
XROWS8 = NV8 + 127


def build_program(mode=MODE):
    import concourse.tile as tile
    from concourse import bacc, mybir
    from bass_rust import VecI64Pair

    f16 = mybir.dt.float16
    f32 = mybir.dt.float32
    f8 = mybir.dt.float8e4
    f8far = mode == "f8far"
    nu16 = 2 if f8far else NTILE    # lag tiles kept fp16

    nc = bacc.Bacc("TRN2", target_bir_lowering=False, debug=False)
    xpad_t = nc.dram_tensor("xpad", [BLOC, XROWS], f16, kind="ExternalInput")
    hcomb_t = nc.dram_tensor("hcomb", [128, M], f16, kind="ExternalInput")
    ucomb_t = nc.dram_tensor("ucomb", [128, nu16 * R], f16, kind="ExternalInput")
    if f8far:
        xpad8_t = nc.dram_tensor("xpad8", [BLOC, XROWS8], f8, kind="ExternalInput")
        ucomb8_t = nc.dram_tensor("ucomb8", [128, 2 * R], f8, kind="ExternalInput")
    y_t = nc.dram_tensor("y", [BLOC, T, M], f16, kind="ExternalOutput")

    with tile.TileContext(nc) as tc:
        with (
            tc.tile_pool(name="xsh", bufs=1) as xsh_pool,
            tc.tile_pool(name="w", bufs=1) as wpool,
            tc.tile_pool(name="psum", bufs=1, space="PSUM") as psum_pool,
            tc.tile_pool(name="stage", bufs=1) as stage_pool,
        ):
            xsh = []
            xsh8 = []
            for b in range(BLOC):
                t_ = xsh_pool.tile([128, NV], f16, tag=f"xshb{b}", name=f"xsh{b}")
                xsh.append(t_)
                if f8far:
                    t8 = xsh_pool.tile(
                        [128, NV8], f8, tag=f"xsh8b{b}", name=f"xsh8{b}"
                    )
                    xsh8.append(t8)
            ucomb_sb = wpool.tile([128, nu16 * R], f16, tag="ucomb", name="ucomb_sb")
            if f8far:
                ucomb8_sb = wpool.tile([128, 2 * R], f8, tag="ucomb8", name="ucomb8_sb")
            hcomb_sb = wpool.tile([128, M], f16, tag="hcomb", name="hcomb_sb")

            def load_xchunk(b, v0, v1, eng):
                in_ap = xpad_t.ap().copy()
                in_ap.ap = VecI64Pair([[1, 128], [1, v1 - v0]])
                in_ap.offset = b * XROWS + v0
                eng.dma_start(out=xsh[b][:, v0:v1], in_=in_ap)

            def load_x8chunk(b, v0, v1, eng):
                in_ap = xpad8_t.ap().copy()
                in_ap.ap = VecI64Pair([[1, 128], [1, v1 - v0]])
                in_ap.offset = b * XROWS8 + v0
                eng.dma_start(out=xsh8[b][:, v0:v1], in_=in_ap)

            # Critical-first load plan.  b0 feeds the first conv groups: its
            # windows go in 512-v slices on the sync HWDGE queue right after
            # tiny ucomb.  b1 rides scalar (free after its ACT_TABLE_LOAD).
            # b2/b3 queue FIFO behind each other on the SWDGE queue -- they
            # aren't touched until the middle wave.
            nc.sync.dma_start(out=ucomb_sb[:], in_=ucomb_t.ap())
            if f8far:
                nc.sync.dma_start(out=ucomb8_sb[:], in_=ucomb8_t.ap())
            nc.scalar.dma_start(out=hcomb_sb[:], in_=hcomb_t.ap())
            for v0 in range(0, NV, 1024):
                load_xchunk(0, v0, min(v0 + 1024, NV), nc.sync)
            for v0 in range(0, NV, 1024):
                load_xchunk(1, v0, min(v0 + 1024, NV), nc.scalar)
            for b in (2, 3):
                for v0 in range(0, NV, 1024):
                    load_xchunk(b, v0, min(v0 + 1024, NV), nc.gpsimd)
            if f8far:
                for b in range(BLOC):
                    for v0 in (0, 1024):
                        load_x8chunk(b, v0, min(v0 + 1024, NV8), nc.gpsimd)

            # ---- compute ----
            evac_engines = [nc.vector, nc.scalar]
            ydma_engines = [nc.sync, nc.gpsimd]
            ci = 0
            yi = 0

            def evac(dst_ap, src_ap):
                nonlocal ci
                eng = evac_engines[ci % 2]
                ci += 1
                if eng is nc.scalar:
                    eng.copy(dst_ap, src_ap)
                else:
                    eng.tensor_copy(dst_ap, src_ap)

            def zcopy(b, s, zt):
                w0 = 512 + 512 * s
                evac(xsh[b][LH:128, w0 : w0 + 512], zt[LH:128, :])

            prevzt = {}

            def conv_group(b, s):
                zt = psum_pool.tile([128, M], f32, tag="zt", bufs=3, name="zt")
                if f8far:
                    # far lag tiles l=3 (i=0) and l=2 (i=1) in one fp8
                    # DoubleRow matmul: rhs is an overlapping (2, 512) view
                    # of the fp8 diag buffer (windows 512s and 512s+128).
                    rhs8 = xsh8[b][:, 0:512].copy()
                    rhs8.ap = VecI64Pair([[NV8, 128], [128, 2], [1, 512]])
                    rhs8.offset = 512 * s
                    nc.tensor.matmul(
                        zt[LH:128, :],
                        lhsT=ucomb8_sb[:].rearrange("p (i q) -> p i q", i=2),
                        rhs=rhs8,
                        start=True,
                        stop=False,
                        perf_mode=mybir.MatmulPerfMode.DoubleRow,
                    )
                    lags = (1, 0)
                else:
                    lags = (3, 2, 1, 0)
                for i, l in enumerate(lags):
                    v = 384 + 512 * s - 128 * l
                    nc.tensor.matmul(
                        zt[LH:128, :],
                        lhsT=ucomb_sb[:, l * R : (l + 1) * R],
                        rhs=xsh[b][:, v : v + 512],
                        start=(i == 0) and not f8far,
                        stop=(l == 0),
                    )
                if s > 0:
                    zcopy(b, s - 1, prevzt[b])
                prevzt[b] = zt
                if s == NSC - 1:
                    zcopy(b, s, zt)

            def out_chunk(b, c, osb, off):
                ot = psum_pool.tile([128, M], f32, tag="ot", bufs=5, name="ot")
                w = 512 + 128 * c
                nc.tensor.matmul(
                    ot[:],
                    lhsT=xsh[b][:, w : w + 128],
                    rhs=hcomb_sb[:],
                    start=True,
                    stop=True,
                )
                evac(osb[:, off : off + M], ot[:])

            def out_group(b, g, ydma_eng=None, nchunks=4, tag="osb", bufs=6):
                nonlocal yi
                osb = stage_pool.tile([128, 4 * M], f16, tag=tag, bufs=bufs, name=tag)
                for k in range(nchunks):
                    out_chunk(b, 4 * g + k, osb, k * M)
                dst = y_t.ap().copy()
                dst.ap = VecI64Pair([[M, 128], [128 * M, nchunks], [1, M]])
                dst.offset = b * T * M + g * 512 * M
                eng = ydma_eng or ydma_engines[yi % 2]
                yi += 1
                eng.dma_start(out=dst, in_=osb[:, : nchunks * M])

            def tail_chunk(b, c, ydma_eng):
                osb = stage_pool.tile([128, 4 * M], f16, tag="osbt", bufs=3, name="osbt")
                out_chunk(b, c, osb, 0)
                dst = y_t.ap().copy()
                dst.ap = VecI64Pair([[M, 128], [128 * M, 1], [1, M]])
                dst.offset = b * T * M + c * 128 * M
                ydma_eng.dma_start(out=dst, in_=osb[:, :M])

            # Schedule.  S1: conv(b0), conv(b1).  W1: out(b0) with conv(b2)
            # interleaved.  W2: out(b1) and out(b2) with conv(b3)
            # interleaved (out(b2, g) is legal once conv(b2, g+1) was
            # emitted, which happened back in W1).  W3: out(b3) with a
            # fine-grained, HWDGE-only tail so the final drain is short.
            for b in (0, 1):
                for s in range(NSC):
                    conv_group(b, s)
            for g in range(4):
                out_group(0, g)
                conv_group(2, g)
            w2 = [
                ("O", 1, 0), ("C", 3, 0), ("O", 2, 0),
                ("O", 1, 1), ("C", 3, 1), ("O", 2, 1),
                ("O", 1, 2), ("C", 3, 2), ("O", 2, 2),
                ("O", 1, 3), ("C", 3, 3), ("O", 2, 3),
            ]
            for kind, b, i in w2:
                if kind == "C":
                    conv_group(b, i)
                else:
                    out_group(b, i)
            out_group(3, 0)
            out_group(3, 1, ydma_eng=nc.sync)
            out_group(3, 2, ydma_eng=nc.scalar)
            out_group(3, 3, ydma_eng=nc.sync, nchunks=2, tag="osbt", bufs=3)
            tail_chunk(3, 14, nc.scalar)
            tail_chunk(3, 15, nc.sync)

    nc.compile()
    return nc


def host_weights(lnl_re, lnl_im, W_r, W_i, C, D, Do, mode=MODE):
    """Impulse response head + SVD-factored tail, float64 math."""
    lnl = lnl_re.astype(np.float64) + 1j * lnl_im.astype(np.float64)
    W = W_r.astype(np.float64) + 1j * W_i.astype(np.float64)
    Winv = np.linalg.inv(W)
    A_re = np.ascontiguousarray(Winv.real.T) @ C.astype(np.float64)
    A_im = np.ascontiguousarray(Winv.imag.T) @ C.astype(np.float64)
    j = np.arange(LH + WT, dtype=np.float64)
    P = np.exp(np.outer(j, lnl))
    H = P.real @ A_re - P.imag @ A_im                 # (LH+WT, M)
    H[0] += D[0].astype(np.float64)

    Hh = H[:LH]
    U, S, Vt = np.linalg.svd(H[LH:], full_matrices=False)
    sq = np.sqrt(S[:R])
    Uf = U[:, :R] * sq                                # (WT, R)
    Vf = sq[:, None] * Vt[:R]                         # (R, M)

    hcomb = np.concatenate([Hh[::-1], Vf], axis=0).astype(np.float16)
    uflip = Uf.reshape(NTILE, 128, R)[:, ::-1, :]     # [l, p, q], p-flipped
    if mode == "f8far":
        import ml_dtypes

        ucomb = (
            uflip[:2].transpose(1, 0, 2).reshape(128, 2 * R).astype(np.float16)
        )
        # DoubleRow weights: i=0 -> lag tile l=3, i=1 -> l=2
        u8 = np.stack([uflip[3], uflip[2]], axis=1).reshape(128, 2 * R)
        return {
            "hcomb": np.ascontiguousarray(hcomb),
            "ucomb": np.ascontiguousarray(ucomb),
            "ucomb8": np.ascontiguousarray(u8.astype(ml_dtypes.float8_e4m3)),
        }
    ucomb = (
        uflip.transpose(1, 0, 2).reshape(128, NTILE * R).astype(np.float16)
    )
    return {
        "hcomb": np.ascontiguousarray(hcomb),
        "ucomb": np.ascontiguousarray(ucomb),
    }


def make_in_maps(x, weights):
    x16 = x[:, :, 0].astype(np.float16)               # (B, T)
    if MODE == "f8far":
        import ml_dtypes

        x8 = x[:, :, 0].astype(ml_dtypes.float8_e4m3)
    in_maps = []
    for c in range(NCORES):
        xpad = np.zeros((BLOC, XROWS), np.float16)
        xpad[:, RPAD : RPAD + T] = x16[c * BLOC : (c + 1) * BLOC]
        im = dict(weights)
        im["xpad"] = xpad
        if MODE == "f8far":
            import ml_dtypes

            xpad8 = np.zeros((BLOC, XROWS8), ml_dtypes.float8_e4m3)
            n8 = XROWS8 - RPAD
            xpad8[:, RPAD:] = x8[c * BLOC : (c + 1) * BLOC, :n8]
            im["xpad8"] = xpad8
        in_maps.append(im)
    return in_maps


_prog_cache = {}


def kernel(x, lnl_re, lnl_im, W_r, W_i, C, D, Do):
    from concourse.bass_utils import run_bass_kernel_spmd

    x = np.asarray(x)
    lnl_re, lnl_im = np.asarray(lnl_re), np.asarray(lnl_im)
    W_r, W_i = np.asarray(W_r), np.asarray(W_i)
    C, D, Do = np.asarray(C), np.asarray(D), np.asarray(Do)

    key = (NLAG, MODE)
    if key not in _prog_cache:
        _prog_cache[key] = build_program()
    nc = _prog_cache[key]

    weights = host_weights(lnl_re, lnl_im, W_r, W_i, C, D, Do)
    in_maps = make_in_maps(x, weights)
    res = run_bass_kernel_spmd(nc, in_maps, core_ids=list(range(NCORES)))
    y = np.concatenate([res.results[i]["y"] for i in range(NCORES)], axis=0)
    y = y.astype(np.float32) + Do.astype(np.float32)[None, None, :]
    return np.ascontiguousarray(y)


# revision 31
# speedup vs baseline: 1.1015x; 1.0159x over previous
"""Trainium2 Bass kernel for nn_LuenbergerLDS (B=32, T=2048, N=512, M=512).

Math: the reference is a diagonal complex linear recurrence followed by a
projection; since d == 1 the whole module is a causal LTI SIMO filter
    y[t, b, m] = sum_{j>=0} H[j, m] * x[t - j, b] + Do[m]
with impulse response H computed on host in float64 from (lam, Winv, C, D).

Structure: the FIR tail H[LH:LH+WT] is numerically low-rank (singular
values fall below 1e-3 of ||y|| by index ~48), so it is factored
H_tail ~= U @ V (rank R = 64) via SVD on host.  The device computes, per
output chunk of 128 timesteps, a SINGLE 128x512 fp16 matmul whose
stationary operand stacks [x head lags (LH=64) ; tail coefficients z
(R=64)] and whose moving operand stacks [H_head ; V].  z comes from a
cheap "basis conv" stage: 4 matmuls per 512-timestep superchunk
contracting lag tiles of U against diagonal (Toeplitz) slices of x.

The z coefficients are written (fp32->fp16 copy, PSUM partitions 64..127
via matmul tile_position) into partitions 64..127 of the SAME per-batch
diagonal x-buffer that serves the head lags in partitions 0..63, so the
output matmul's stationary operand is a plain contiguous slice.
ORDERING HAZARD: that overlay write lands on cells conv(b, s+1) still
reads as x data, so zcopy(b, s) must be emitted after conv(b, s+1); the
zt PSUM pool needs 3 buffers so the slot-reuser group sits 2 groups
behind the zcopy and the WAR wait never stalls the PE.

dtype fp16 on the PE (measured end-to-end error 6.2e-4 of max|y| vs
2e-2 tolerance); PSUM accumulates fp32; y is written fp16 and upcast on
host (Do added there).  Per core: 64 conv + 64 output matmuls.

Engine economy: only DVE + Activation can read PSUM, so evacuation
copies rotate across exactly those two; y writes are staged 4 chunks
per dma_start and issued from SP/Pool; loads are ordered critical-first
(b0/b1 ride the two HWDGE queues; b2/b3 queue FIFO behind on SWDGE).
"""

import os
import sys

sys.path.insert(0, "/opt/trn_rl_repo")

import numpy as np

# problem dims (hardcoded per harness contract)
B, T, N, M = 32, 2048, 512, 512
NCORES = 8
BLOC = B // NCORES          # batches per core
LH = 64                     # direct head lags [0, LH); partition-quadrant aligned
R = 128 - LH                # tail rank (stacked into the same 128 contraction)
NTILE = 4                   # conv lag tiles of 128 -> tail window
WT = NTILE * 128            # tail lags [LH, LH+WT)
RPAD = LH + WT - 1          # 575: left zero pad of x
NV = T + 512                # diag buffer v-range [0, NV)
XROWS = NV + 127            # xpad rows so the diag load never reads OOB
NCH = T // 128              # output chunks per batch
NSC = T // 512              # superchunks per batch

MODE = os.environ.get("K_MODE", "f16")  # f16 | f8far
NLAG = NTILE  # kept for test.py cache-key compatibility
NV8 = 2176                  # fp8 far-tail diag buffer v-range (f8far mode)
XROWS8 = NV8 + 127


def build_program(mode=MODE):
    import concourse.tile as tile
    from concourse import bacc, mybir
    from bass_rust import VecI64Pair

    f16 = mybir.dt.float16
    f32 = mybir.dt.float32
    f8 = mybir.dt.float8e4
    f8far = mode == "f8far"
    nu16 = 2 if f8far else NTILE    # lag tiles kept fp16

    nc = bacc.Bacc("TRN2", target_bir_lowering=False, debug=False)
    xpad_t = nc.dram_tensor("xpad", [BLOC, XROWS], f16, kind="ExternalInput")
    hcomb_t = nc.dram_tensor("hcomb", [128, M], f16, kind="ExternalInput")
    ucomb_t = nc.dram_tensor("ucomb", [128, nu16 * R], f16, kind="ExternalInput")
    if f8far:
        xpad8_t = nc.dram_tensor("xpad8", [BLOC, XROWS8], f8, kind="ExternalInput")
        ucomb8_t = nc.dram_tensor("ucomb8", [128, 2 * R], f8, kind="ExternalInput")
    y_t = nc.dram_tensor("y", [BLOC, T, M], f16, kind="ExternalOutput")

    with tile.TileContext(nc) as tc:
        with (
            tc.tile_pool(name="xsh", bufs=1) as xsh_pool,
            tc.tile_pool(name="w", bufs=1) as wpool,
            tc.tile_pool(name="psum", bufs=1, space="PSUM") as psum_pool,
            tc.tile_pool(name="stage", bufs=1) as stage_pool,
        ):
            xsh = []
            xsh8 = []
            for b in range(BLOC):
                t_ = xsh_pool.tile([128, NV], f16, tag=f"xshb{b}", name=f"xsh{b}")
                xsh.append(t_)
                if f8far:
                    t8 = xsh_pool.tile(
                        [128, NV8], f8, tag=f"xsh8b{b}", name=f"xsh8{b}"
                    )
                    xsh8.append(t8)
            ucomb_sb = wpool.tile([128, nu16 * R], f16, tag="ucomb", name="ucomb_sb")
            if f8far:
                ucomb8_sb = wpool.tile([128, 2 * R], f8, tag="ucomb8", name="ucomb8_sb")
            hcomb_sb = wpool.tile([128, M], f16, tag="hcomb", name="hcomb_sb")

            def load_xchunk(b, v0, v1, eng):
                in_ap = xpad_t.ap().copy()
                in_ap.ap = VecI64Pair([[1, 128], [1, v1 - v0]])
                in_ap.offset = b * XROWS + v0
                eng.dma_start(out=xsh[b][:, v0:v1], in_=in_ap)

            def load_x8chunk(b, v0, v1, eng):
                in_ap = xpad8_t.ap().copy()
                in_ap.ap = VecI64Pair([[1, 128], [1, v1 - v0]])
                in_ap.offset = b * XROWS8 + v0
                eng.dma_start(out=xsh8[b][:, v0:v1], in_=in_ap)

            # Critical-first load plan.  b0 feeds the first conv groups: its
            # windows go in 512-v slices on the sync HWDGE queue right after
            # tiny ucomb.  b1 rides scalar (free after its ACT_TABLE_LOAD).
            # b2/b3 queue FIFO behind each other on the SWDGE queue -- they
            # aren't touched until the middle wave.
            nc.sync.dma_start(out=ucomb_sb[:], in_=ucomb_t.ap())
            if f8far:
                nc.sync.dma_start(out=ucomb8_sb[:], in_=ucomb8_t.ap())
            nc.scalar.dma_start(out=hcomb_sb[:], in_=hcomb_t.ap())
            for v0 in range(0, NV, 1024):
                load_xchunk(0, v0, min(v0 + 1024, NV), nc.sync)
            for v0 in range(0, NV, 1024):
                load_xchunk(1, v0, min(v0 + 1024, NV), nc.scalar)
            for b in (2, 3):
                for v0 in range(0, NV, 1024):
                    load_xchunk(b, v0, min(v0 + 1024, NV), nc.gpsimd)
            if f8far:
                for b in range(BLOC):
                    for v0 in (0, 1024):
                        load_x8chunk(b, v0, min(v0 + 1024, NV8), nc.gpsimd)

            # ---- compute ----
            evac_engines = [nc.vector, nc.scalar]
            ydma_engines = [nc.gpsimd, nc.sync]
            ci = 0
            yi = 0

            def evac(dst_ap, src_ap):
                nonlocal ci
                eng = evac_engines[ci % 2]
                ci += 1
                if eng is nc.scalar:
                    eng.copy(dst_ap, src_ap)
                else:
                    eng.tensor_copy(dst_ap, src_ap)

            def zcopy(b, s, zt):
                w0 = 512 + 512 * s
                evac(xsh[b][LH:128, w0 : w0 + 512], zt[LH:128, :])

            prevzt = {}

            def conv_group(b, s):
                zt = psum_pool.tile([128, M], f32, tag="zt", bufs=3, name="zt")
                if f8far:
                    # far lag tiles l=3 (i=0) and l=2 (i=1) in one fp8
                    # DoubleRow matmul: rhs is an overlapping (2, 512) view
                    # of the fp8 diag buffer (windows 512s and 512s+128).
                    rhs8 = xsh8[b][:, 0:512].copy()
                    rhs8.ap = VecI64Pair([[NV8, 128], [128, 2], [1, 512]])
                    rhs8.offset = 512 * s
                    nc.tensor.matmul(
                        zt[LH:128, :],
                        lhsT=ucomb8_sb[:].rearrange("p (i q) -> p i q", i=2),
                        rhs=rhs8,
                        start=True,
                        stop=False,
                        perf_mode=mybir.MatmulPerfMode.DoubleRow,
                    )
                    lags = (1, 0)
                else:
                    lags = (3, 2, 1, 0)
                for i, l in enumerate(lags):
                    v = 384 + 512 * s - 128 * l
                    nc.tensor.matmul(
                        zt[LH:128, :],
                        lhsT=ucomb_sb[:, l * R : (l + 1) * R],
                        rhs=xsh[b][:, v : v + 512],
                        start=(i == 0) and not f8far,
                        stop=(l == 0),
                    )
                if s > 0:
                    zcopy(b, s - 1, prevzt[b])
                prevzt[b] = zt
                if s == NSC - 1:
                    zcopy(b, s, zt)

            def out_chunk(b, c, osb, off):
                ot = psum_pool.tile([128, M], f32, tag="ot", bufs=5, name="ot")
                w = 512 + 128 * c
                nc.tensor.matmul(
                    ot[:],
                    lhsT=xsh[b][:, w : w + 128],
                    rhs=hcomb_sb[:],
                    start=True,
                    stop=True,
                )
                evac(osb[:, off : off + M], ot[:])

            def out_group(b, g, ydma_eng=None, nchunks=4, tag="osb", bufs=6):
                nonlocal yi
                osb = stage_pool.tile([128, 4 * M], f16, tag=tag, bufs=bufs, name=tag)
                for k in range(nchunks):
                    out_chunk(b, 4 * g + k, osb, k * M)
                dst = y_t.ap().copy()
                dst.ap = VecI64Pair([[M, 128], [128 * M, nchunks], [1, M]])
                dst.offset = b * T * M + g * 512 * M
                eng = ydma_eng or ydma_engines[yi % 2]
                yi += 1
                eng.dma_start(out=dst, in_=osb[:, : nchunks * M])

            def tail_chunk(b, c, ydma_eng, evac_eng):
                osb = stage_pool.tile([128, 4 * M], f16, tag="osbt", bufs=3, name="osbt")
                ot = psum_pool.tile([128, M], f32, tag="ot", bufs=5, name="ot")
                w = 512 + 128 * c
                nc.tensor.matmul(
                    ot[:], lhsT=xsh[b][:, w : w + 128], rhs=hcomb_sb[:],
                    start=True, stop=True,
                )
                if evac_eng is nc.scalar:
                    evac_eng.copy(osb[:, :M], ot[:])
                else:
                    evac_eng.tensor_copy(osb[:, :M], ot[:])
                dst = y_t.ap().copy()
                dst.ap = VecI64Pair([[M, 128], [128 * M, 1], [1, M]])
                dst.offset = b * T * M + c * 128 * M
                ydma_eng.dma_start(out=dst, in_=osb[:, :M])

            # Schedule.  S1: conv(b0), conv(b1).  W1: out(b0) with conv(b2)
            # interleaved.  W2: out(b1) and out(b2) with conv(b3)
            # interleaved (out(b2, g) is legal once conv(b2, g+1) was
            # emitted, which happened back in W1).  W3: out(b3) with a
            # fine-grained, HWDGE-only tail so the final drain is short.
            for b in (0, 1):
                for s in range(NSC):
                    conv_group(b, s)
            for g in range(4):
                out_group(0, g)
                conv_group(2, g)
            w2 = [
                ("O", 1, 0), ("C", 3, 0), ("O", 2, 0),
                ("O", 1, 1), ("C", 3, 1), ("O", 2, 1),
                ("O", 1, 2), ("C", 3, 2), ("O", 2, 2),
                ("O", 1, 3), ("C", 3, 3), ("O", 2, 3),
            ]
            for kind, b, i in w2:
                if kind == "C":
                    conv_group(b, i)
                else:
                    out_group(b, i)
            out_group(3, 0)
            out_group(3, 1, ydma_eng=nc.sync)
            out_group(3, 2, ydma_eng=nc.gpsimd)
            out_group(3, 3, ydma_eng=nc.sync, nchunks=2, tag="osbt", bufs=3)
            tail_chunk(3, 14, nc.sync, nc.scalar)
            tail_chunk(3, 15, nc.sync, nc.vector)

    nc.compile()
    return nc


def host_weights(lnl_re, lnl_im, W_r, W_i, C, D, Do, mode=MODE):
    """Impulse response head + SVD-factored tail, float64 math."""
    lnl = lnl_re.astype(np.float64) + 1j * lnl_im.astype(np.float64)
    W = W_r.astype(np.float64) + 1j * W_i.astype(np.float64)
    Winv = np.linalg.inv(W)
    A_re = np.ascontiguousarray(Winv.real.T) @ C.astype(np.float64)
    A_im = np.ascontiguousarray(Winv.imag.T) @ C.astype(np.float64)
    j = np.arange(LH + WT, dtype=np.float64)
    P = np.exp(np.outer(j, lnl))
    H = P.real @ A_re - P.imag @ A_im                 # (LH+WT, M)
    H[0] += D[0].astype(np.float64)

    Hh = H[:LH]
    U, S, Vt = np.linalg.svd(H[LH:], full_matrices=False)
    sq = np.sqrt(S[:R])
    Uf = U[:, :R] * sq                                # (WT, R)
    Vf = sq[:, None] * Vt[:R]                         # (R, M)

    hcomb = np.concatenate([Hh[::-1], Vf], axis=0).astype(np.float16)
    uflip = Uf.reshape(NTILE, 128, R)[:, ::-1, :]     # [l, p, q], p-flipped
    if mode == "f8far":
        import ml_dtypes

        ucomb = (
            uflip[:2].transpose(1, 0, 2).reshape(128, 2 * R).astype(np.float16)
        )
        # DoubleRow weights: i=0 -> lag tile l=3, i=1 -> l=2
        u8 = np.stack([uflip[3], uflip[2]], axis=1).reshape(128, 2 * R)
        return {
            "hcomb": np.ascontiguousarray(hcomb),
            "ucomb": np.ascontiguousarray(ucomb),
            "ucomb8": np.ascontiguousarray(u8.astype(ml_dtypes.float8_e4m3)),
        }
    ucomb = (
        uflip.transpose(1, 0, 2).reshape(128, NTILE * R).astype(np.float16)
    )
    return {
        "hcomb": np.ascontiguousarray(hcomb),
        "ucomb": np.ascontiguousarray(ucomb),
    }


def make_in_maps(x, weights):
    x16 = x[:, :, 0].astype(np.float16)               # (B, T)
    if MODE == "f8far":
        import ml_dtypes

        x8 = x[:, :, 0].astype(ml_dtypes.float8_e4m3)
    in_maps = []
    for c in range(NCORES):
        xpad = np.zeros((BLOC, XROWS), np.float16)
        xpad[:, RPAD : RPAD + T] = x16[c * BLOC : (c + 1) * BLOC]
        im = dict(weights)
        im["xpad"] = xpad
        if MODE == "f8far":
            import ml_dtypes

            xpad8 = np.zeros((BLOC, XROWS8), ml_dtypes.float8_e4m3)
            n8 = XROWS8 - RPAD
            xpad8[:, RPAD:] = x8[c * BLOC : (c + 1) * BLOC, :n8]
            im["xpad8"] = xpad8
        in_maps.append(im)
    return in_maps


_prog_cache = {}


def kernel(x, lnl_re, lnl_im, W_r, W_i, C, D, Do):
    from concourse.bass_utils import run_bass_kernel_spmd

    x = np.asarray(x)
    lnl_re, lnl_im = np.asarray(lnl_re), np.asarray(lnl_im)
    W_r, W_i = np.asarray(W_r), np.asarray(W_i)
    C, D, Do = np.asarray(C), np.asarray(D), np.asarray(Do)

    key = (NLAG, MODE)
    if key not in _prog_cache:
        _prog_cache[key] = build_program()
    nc = _prog_cache[key]

    weights = host_weights(lnl_re, lnl_im, W_r, W_i, C, D, Do)
    in_maps = make_in_maps(x, weights)
    res = run_bass_kernel_spmd(nc, in_maps, core_ids=list(range(NCORES)))
    y = np.concatenate([res.results[i]["y"] for i in range(NCORES)], axis=0)
    y = y.astype(np.float32) + Do.astype(np.float32)[None, None, :]
    return np.ascontiguousarray(y)


# revision 33
# speedup vs baseline: 1.1542x; 1.0479x over previous
"""Trainium2 Bass kernel for nn_LuenbergerLDS (B=32, T=2048, N=512, M=512).

Math: the reference is a diagonal complex linear recurrence followed by a
projection; since d == 1 the whole module is a causal LTI SIMO filter
    y[t, b, m] = sum_{j>=0} H[j, m] * x[t - j, b] + Do[m]
with impulse response H computed on host in float64 from (lam, Winv, C, D).

Structure: the FIR tail H[LH:LH+WT] is numerically low-rank (singular
values fall below 1e-3 of ||y|| by index ~48), so it is factored
H_tail ~= U @ V (rank R = 64) via SVD on host.  The device computes, per
output chunk of 128 timesteps, a SINGLE 128x512 fp16 matmul whose
stationary operand stacks [x head lags (LH=64) ; tail coefficients z
(R=64)] and whose moving operand stacks [H_head ; V].  z comes from a
cheap "basis conv" stage: 4 matmuls per 512-timestep superchunk
contracting lag tiles of U against diagonal (Toeplitz) slices of x.

The z coefficients are written (fp32->fp16 copy, PSUM partitions 64..127
via matmul tile_position) into partitions 64..127 of the SAME per-batch
diagonal x-buffer that serves the head lags in partitions 0..63, so the
output matmul's stationary operand is a plain contiguous slice.
ORDERING HAZARD: that overlay write lands on cells conv(b, s+1) still
reads as x data, so zcopy(b, s) must be emitted after conv(b, s+1); the
zt PSUM pool needs 3 buffers so the slot-reuser group sits 2 groups
behind the zcopy and the WAR wait never stalls the PE.

dtype fp16 on the PE (measured end-to-end error 6.2e-4 of max|y| vs
2e-2 tolerance); PSUM accumulates fp32; y is written fp16 and upcast on
host (Do added there).  Per core: 64 conv + 64 output matmuls.

Engine economy: only DVE + Activation can read PSUM, so evacuation
copies rotate across exactly those two; y writes are staged 4 chunks
per dma_start and issued from SP/Pool; loads are ordered critical-first
(b0/b1 ride the two HWDGE queues; b2/b3 queue FIFO behind on SWDGE).
"""

import os
import sys

sys.path.insert(0, "/opt/trn_rl_repo")

import numpy as np

# problem dims (hardcoded per harness contract)
B, T, N, M = 32, 2048, 512, 512
NCORES = 8
BLOC = B // NCORES          # batches per core
LH = 64                     # direct head lags [0, LH); partition-quadrant aligned
R = 128 - LH                # tail rank (stacked into the same 128 contraction)
NTILE = 3                   # conv lag tiles of 128 -> tail window
WT = NTILE * 128            # tail lags [LH, LH+WT)
RPAD = LH + WT - 1          # 575: left zero pad of x
NV = T + WT                 # diag buffer v-range [0, NV)
XROWS = NV + 127            # xpad rows so the diag load never reads OOB
NCH = T // 128              # output chunks per batch
NSC = T // 512              # superchunks per batch

MODE = os.environ.get("K_MODE", "f16")  # f16 | f8far
NLAG = NTILE  # kept for test.py cache-key compatibility
NV8 = 2176                  # fp8 far-tail diag buffer v-range (f8far mode)
XROWS8 = NV8 + 127


def build_program(mode=MODE):
    import concourse.tile as tile
    from concourse import bacc, mybir
    from bass_rust import VecI64Pair

    f16 = mybir.dt.float16
    f32 = mybir.dt.float32
    f8 = mybir.dt.float8e4
    f8far = mode == "f8far"
    nu16 = 2 if f8far else NTILE    # lag tiles kept fp16

    nc = bacc.Bacc("TRN2", target_bir_lowering=False, debug=False)
    xpad_t = nc.dram_tensor("xpad", [BLOC, XROWS], f16, kind="ExternalInput")
    hcomb_t = nc.dram_tensor("hcomb", [128, M], f16, kind="ExternalInput")
    ucomb_t = nc.dram_tensor("ucomb", [128, nu16 * R], f16, kind="ExternalInput")
    if f8far:
        xpad8_t = nc.dram_tensor("xpad8", [BLOC, XROWS8], f8, kind="ExternalInput")
        ucomb8_t = nc.dram_tensor("ucomb8", [128, 2 * R], f8, kind="ExternalInput")
    y_t = nc.dram_tensor("y", [BLOC, T, M], f16, kind="ExternalOutput")

    with tile.TileContext(nc) as tc:
        with (
            tc.tile_pool(name="xsh", bufs=1) as xsh_pool,
            tc.tile_pool(name="w", bufs=1) as wpool,
            tc.tile_pool(name="psum", bufs=1, space="PSUM") as psum_pool,
            tc.tile_pool(name="stage", bufs=1) as stage_pool,
        ):
            xsh = []
            xsh8 = []
            for b in range(BLOC):
                t_ = xsh_pool.tile([128, NV], f16, tag=f"xshb{b}", name=f"xsh{b}")
                xsh.append(t_)
                if f8far:
                    t8 = xsh_pool.tile(
                        [128, NV8], f8, tag=f"xsh8b{b}", name=f"xsh8{b}"
                    )
                    xsh8.append(t8)
            ucomb_sb = wpool.tile([128, nu16 * R], f16, tag="ucomb", name="ucomb_sb")
            if f8far:
                ucomb8_sb = wpool.tile([128, 2 * R], f8, tag="ucomb8", name="ucomb8_sb")
            hcomb_sb = wpool.tile([128, M], f16, tag="hcomb", name="hcomb_sb")

            def load_xchunk(b, v0, v1, eng):
                in_ap = xpad_t.ap().copy()
                in_ap.ap = VecI64Pair([[1, 128], [1, v1 - v0]])
                in_ap.offset = b * XROWS + v0
                eng.dma_start(out=xsh[b][:, v0:v1], in_=in_ap)

            def load_x8chunk(b, v0, v1, eng):
                in_ap = xpad8_t.ap().copy()
                in_ap.ap = VecI64Pair([[1, 128], [1, v1 - v0]])
                in_ap.offset = b * XROWS8 + v0
                eng.dma_start(out=xsh8[b][:, v0:v1], in_=in_ap)

            # Critical-first load plan.  b0 feeds the first conv groups: its
            # windows go in 512-v slices on the sync HWDGE queue right after
            # tiny ucomb.  b1 rides scalar (free after its ACT_TABLE_LOAD).
            # b2/b3 queue FIFO behind each other on the SWDGE queue -- they
            # aren't touched until the middle wave.
            nc.sync.dma_start(out=ucomb_sb[:], in_=ucomb_t.ap())
            if f8far:
                nc.sync.dma_start(out=ucomb8_sb[:], in_=ucomb8_t.ap())
            nc.scalar.dma_start(out=hcomb_sb[:], in_=hcomb_t.ap())
            for v0 in range(0, NV, 1024):
                load_xchunk(0, v0, min(v0 + 1024, NV), nc.sync)
            for v0 in range(0, NV, 1024):
                load_xchunk(1, v0, min(v0 + 1024, NV), nc.scalar)
            for b in (2, 3):
                for v0 in range(0, NV, 1024):
                    load_xchunk(b, v0, min(v0 + 1024, NV), nc.gpsimd)
            if f8far:
                for b in range(BLOC):
                    for v0 in (0, 1024):
                        load_x8chunk(b, v0, min(v0 + 1024, NV8), nc.gpsimd)

            # ---- compute ----
            evac_engines = [nc.vector, nc.scalar]
            ydma_engines = [nc.gpsimd, nc.sync]
            ci = 0
            yi = 0

            def evac(dst_ap, src_ap):
                nonlocal ci
                eng = evac_engines[ci % 2]
                ci += 1
                if eng is nc.scalar:
                    eng.copy(dst_ap, src_ap)
                else:
                    eng.tensor_copy(dst_ap, src_ap)

            # In f8far mode the DoubleRow matmul requires dst partition
            # base 0, so z lives at partitions 0..63 and the head lags at
            # 64..127 (diag windows shift by -64; hcomb rows swap on host).
            ZLO, ZHI = (0, R) if f8far else (LH, 128)
            W0 = (WT - 64) if f8far else WT

            def zcopy(b, s, zt):
                w0 = W0 + 512 * s
                evac(xsh[b][ZLO:ZHI, w0 : w0 + 512], zt[ZLO:ZHI, :])

            prevzt = {}

            def conv_group(b, s):
                zt = psum_pool.tile([128, M], f32, tag="zt", bufs=3, name="zt")
                if f8far:
                    # far lag tiles l=3 (i=0) and l=2 (i=1) in one fp8
                    # DoubleRow matmul: rhs is an overlapping (2, 512) view
                    # of the fp8 diag buffer (windows 512s and 512s+128).
                    rhs8 = xsh8[b][:, 0:512].copy()
                    rhs8.ap = VecI64Pair([[NV8, 128], [128, 2], [1, 512]])
                    rhs8.offset = 512 * s
                    nc.tensor.matmul(
                        zt[ZLO:ZHI, :],
                        lhsT=ucomb8_sb[:].rearrange("p (i q) -> p i q", i=2),
                        rhs=rhs8,
                        start=True,
                        stop=False,
                        perf_mode=mybir.MatmulPerfMode.DoubleRow,
                    )
                    lags = (1, 0)
                else:
                    lags = tuple(range(NTILE - 1, -1, -1))
                for i, l in enumerate(lags):
                    v = (WT - 128) + 512 * s - 128 * l
                    nc.tensor.matmul(
                        zt[ZLO:ZHI, :],
                        lhsT=ucomb_sb[:, l * R : (l + 1) * R],
                        rhs=xsh[b][:, v : v + 512],
                        start=(i == 0) and not f8far,
                        stop=(l == 0),
                    )
                if s > 0:
                    zcopy(b, s - 1, prevzt[b])
                prevzt[b] = zt
                if s == NSC - 1:
                    zcopy(b, s, zt)

            def out_chunk(b, c, osb, off):
                ot = psum_pool.tile([128, M], f32, tag="ot", bufs=5, name="ot")
                w = W0 + 128 * c
                nc.tensor.matmul(
                    ot[:],
                    lhsT=xsh[b][:, w : w + 128],
                    rhs=hcomb_sb[:],
                    start=True,
                    stop=True,
                )
                evac(osb[:, off : off + M], ot[:])

            def out_group(b, g, ydma_eng=None, nchunks=4, tag="osb", bufs=6):
                nonlocal yi
                osb = stage_pool.tile([128, 4 * M], f16, tag=tag, bufs=bufs, name=tag)
                for k in range(nchunks):
                    out_chunk(b, 4 * g + k, osb, k * M)
                dst = y_t.ap().copy()
                dst.ap = VecI64Pair([[M, 128], [128 * M, nchunks], [1, M]])
                dst.offset = b * T * M + g * 512 * M
                eng = ydma_eng or ydma_engines[yi % 2]
                yi += 1
                eng.dma_start(out=dst, in_=osb[:, : nchunks * M])

            def tail_chunk(b, c, ydma_eng, evac_eng):
                osb = stage_pool.tile([128, 4 * M], f16, tag="osbt", bufs=3, name="osbt")
                ot = psum_pool.tile([128, M], f32, tag="ot", bufs=5, name="ot")
                w = W0 + 128 * c
                nc.tensor.matmul(
                    ot[:], lhsT=xsh[b][:, w : w + 128], rhs=hcomb_sb[:],
                    start=True, stop=True,
                )
                if evac_eng is nc.scalar:
                    evac_eng.copy(osb[:, :M], ot[:])
                else:
                    evac_eng.tensor_copy(osb[:, :M], ot[:])
                dst = y_t.ap().copy()
                dst.ap = VecI64Pair([[M, 128], [128 * M, 1], [1, M]])
                dst.offset = b * T * M + c * 128 * M
                ydma_eng.dma_start(out=dst, in_=osb[:, :M])

            # Schedule.  S1: conv(b0), conv(b1).  W1: out(b0) with conv(b2)
            # interleaved.  W2: out(b1) and out(b2) with conv(b3)
            # interleaved (out(b2, g) is legal once conv(b2, g+1) was
            # emitted, which happened back in W1).  W3: out(b3) with a
            # fine-grained, HWDGE-only tail so the final drain is short.
            for b in (0, 1):
                for s in range(NSC):
                    conv_group(b, s)
            for g in range(4):
                out_group(0, g)
                conv_group(2, g)
            w2 = [
                ("O", 1, 0), ("C", 3, 0), ("O", 2, 0),
                ("O", 1, 1), ("C", 3, 1), ("O", 2, 1),
                ("O", 1, 2), ("C", 3, 2), ("O", 2, 2),
                ("O", 1, 3), ("C", 3, 3), ("O", 2, 3),
            ]
            for kind, b, i in w2:
                if kind == "C":
                    conv_group(b, i)
                else:
                    out_group(b, i)
            out_group(3, 0)
            out_group(3, 1, ydma_eng=nc.sync)
            out_group(3, 2, ydma_eng=nc.gpsimd)
            out_group(3, 3, ydma_eng=nc.sync, nchunks=2, tag="osbt", bufs=3)
            tail_chunk(3, 14, nc.sync, nc.scalar)
            tail_chunk(3, 15, nc.sync, nc.vector)

    nc.compile()
    return nc


def host_weights(lnl_re, lnl_im, W_r, W_i, C, D, Do, mode=MODE):
    """Impulse response head + SVD-factored tail, float64 math."""
    lnl = lnl_re.astype(np.float64) + 1j * lnl_im.astype(np.float64)
    W = W_r.astype(np.float64) + 1j * W_i.astype(np.float64)
    Winv = np.linalg.inv(W)
    A_re = np.ascontiguousarray(Winv.real.T) @ C.astype(np.float64)
    A_im = np.ascontiguousarray(Winv.imag.T) @ C.astype(np.float64)
    j = np.arange(LH + WT, dtype=np.float64)
    P = np.exp(np.outer(j, lnl))
    H = P.real @ A_re - P.imag @ A_im                 # (LH+WT, M)
    H[0] += D[0].astype(np.float64)

    Hh = H[:LH]
    U, S, Vt = np.linalg.svd(H[LH:], full_matrices=False)
    sq = np.sqrt(S[:R])
    Uf = U[:, :R] * sq                                # (WT, R)
    Vf = sq[:, None] * Vt[:R]                         # (R, M)

    if mode == "f8far":
        hcomb = np.concatenate([Vf, Hh[::-1]], axis=0).astype(np.float16)
    else:
        hcomb = np.concatenate([Hh[::-1], Vf], axis=0).astype(np.float16)
    uflip = Uf.reshape(NTILE, 128, R)[:, ::-1, :]     # [l, p, q], p-flipped
    if mode == "f8far":
        import ml_dtypes

        ucomb = (
            uflip[:2].transpose(1, 0, 2).reshape(128, 2 * R).astype(np.float16)
        )
        # DoubleRow weights: i=0 -> lag tile l=3, i=1 -> l=2
        u8 = np.stack([uflip[3], uflip[2]], axis=1).reshape(128, 2 * R)
        return {
            "hcomb": np.ascontiguousarray(hcomb),
            "ucomb": np.ascontiguousarray(ucomb),
            "ucomb8": np.ascontiguousarray(u8.astype(ml_dtypes.float8_e4m3)),
        }
    ucomb = (
        uflip.transpose(1, 0, 2).reshape(128, NTILE * R).astype(np.float16)
    )
    return {
        "hcomb": np.ascontiguousarray(hcomb),
        "ucomb": np.ascontiguousarray(ucomb),
    }


def make_in_maps(x, weights):
    x16 = x[:, :, 0].astype(np.float16)               # (B, T)
    if MODE == "f8far":
        import ml_dtypes

        x8 = x[:, :, 0].astype(ml_dtypes.float8_e4m3)
    in_maps = []
    for c in range(NCORES):
        xpad = np.zeros((BLOC, XROWS), np.float16)
        xpad[:, RPAD : RPAD + T] = x16[c * BLOC : (c + 1) * BLOC]
        im = dict(weights)
        im["xpad"] = xpad
        if MODE == "f8far":
            import ml_dtypes

            xpad8 = np.zeros((BLOC, XROWS8), ml_dtypes.float8_e4m3)
            n8 = XROWS8 - RPAD
            xpad8[:, RPAD:] = x8[c * BLOC : (c + 1) * BLOC, :n8]
            im["xpad8"] = xpad8
        in_maps.append(im)
    return in_maps


_prog_cache = {}


def kernel(x, lnl_re, lnl_im, W_r, W_i, C, D, Do):
    from concourse.bass_utils import run_bass_kernel_spmd

    x = np.asarray(x)
    lnl_re, lnl_im = np.asarray(lnl_re), np.asarray(lnl_im)
    W_r, W_i = np.asarray(W_r), np.asarray(W_i)
    C, D, Do = np.asarray(C), np.asarray(D), np.asarray(Do)

    key = (NLAG, MODE)
    if key not in _prog_cache:
        _prog_cache[key] = build_program()
    nc = _prog_cache[key]

    weights = host_weights(lnl_re, lnl_im, W_r, W_i, C, D, Do)
    in_maps = make_in_maps(x, weights)
    res = run_bass_kernel_spmd(nc, in_maps, core_ids=list(range(NCORES)))
    y = np.concatenate([res.results[i]["y"] for i in range(NCORES)], axis=0)
    y = y.astype(np.float32) + Do.astype(np.float32)[None, None, :]
    return np.ascontiguousarray(y)
